# revision 1
# baseline (speedup 1.0000x reference)
"""Fused varlen SigLIP attention block for TRN2, tensor-parallel over heads
across 8 NeuronCores (2 heads per core).

Per core (heads 2c, 2c+1):
  - host pre-transposes x -> xT [H, T] (bf16); all matmuls stream xT.
  - qkvT per t-tile: psum[tl, 432] = xT_tile.T @ wqkvT  (cols q0 q1 k0 k1 v0 v1)
  - rope in t-major on strided [4, 36] half views; PE-transpose q,k per
    72-col tensor into QK [72, 4, T] bf16 (d-major).
  - v evacuated s-major into vseg [tl, 146] = [v0|1|v1|1]; the ones column
    makes the PV matmul emit the softmax row-sum as ctx row 72.
  - per segment/chunk(<=512)/s-tile(<=128): scoresT=kT.T@qT, exp on ACT
    (scale=1/sqrt(72), bias=-4 cancels in normalization), PV accumulates
    ctxT[73, tn]; normalize by DMA-broadcast 1/rowsum.
  - out-proj row-parallel: outT_partial[1152, T]; host sums the 8 partials.

Tiling is segment-aligned from cu_seqlens values (any sorted values work);
the BIR is specialized per plan and cached. bqkv/bout are zeros per spec;
bout is still added on the host.
"""
import numpy as np
from contextlib import ExitStack

import ml_dtypes
import concourse.bass as bass
import concourse.bacc as bacc
import concourse.tile as tile
import concourse.mybir as mybir
from concourse import bass_utils

F32 = mybir.dt.float32
BF16 = mybir.dt.bfloat16

H = 1152
NH = 16
HD = 72
HD2 = 36
T = 4096
NCORES = 8
HPC = NH // NCORES          # heads per core
OC = 3 * HPC * HD           # 432
SCALE = HD ** -0.5
EXP_BIAS = -4.0

_CACHE = {}


def _plan(cu):
    bs = sorted(set([0, T] + [int(v) for v in cu[1:] if 0 < int(v) < T]))
    segs = [(a, b) for a, b in zip(bs[:-1], bs[1:]) if b > a]
    plan = []
    for (a, b) in segs:
        chunks = []
        c0 = a
        while c0 < b:
            cn = min(512, b - c0)
            tls = []
            t0 = c0
            while t0 < c0 + cn:
                tl = min(128, c0 + cn - t0)
                tls.append((t0, tl))
                t0 += tl
            chunks.append((c0, cn, tuple(tls)))
            c0 += cn
        plan.append((a, b, tuple(chunks)))
    return tuple(plan)


def _all_tiles(plan):
    out = []
    for a, b, chunks in plan:
        for c0, cn, tls in chunks:
            out.extend(tls)
    return out


def build(nc, plan):
    tiles = _all_tiles(plan)
    nt = len(tiles)
    tidx = {t0: i for i, (t0, tl) in enumerate(tiles)}

    x_t = nc.dram_tensor("x_t", [H, T], BF16, kind="ExternalInput").ap()
    wq_t = nc.dram_tensor("wq_t", [H, OC], BF16, kind="ExternalInput").ap()
    wo_t = nc.dram_tensor("wo_t", [HPC, HD, H], BF16, kind="ExternalInput").ap()
    cs4d = nc.dram_tensor("cs4d", [nt, 128, 288], F32, kind="ExternalInput").ap()
    idd = nc.dram_tensor("idd", [128, 128], BF16, kind="ExternalInput").ap()
    outT = nc.dram_tensor("outT", [H, T], F32, kind="ExternalOutput").ap()

    with tile.TileContext(nc) as tc, ExitStack() as ctx:
        P = lambda **kw: ctx.enter_context(tc.tile_pool(**kw))
        singles = P(name="singles", bufs=1)
        xin = P(name="xin", bufs=2)
        stp = P(name="stp", bufs=3)
        tmp = P(name="tmp", bufs=2)
        esp = P(name="esp", bufs=3)
        cxp = P(name="cxp", bufs=5)
        bcp = P(name="bcp", bufs=2)
        osb = P(name="osb", bufs=4)
        ps_qkv = P(name="ps_qkv", bufs=2, space="PSUM")
        ps_tp = P(name="ps_tp", bufs=1, space="PSUM")
        ps_sc = P(name="ps_sc", bufs=3, space="PSUM")
        ps_cx = P(name="ps_cx", bufs=1, space="PSUM")
        ps_ou = P(name="ps_ou", bufs=1, space="PSUM")

        wq_sb = singles.tile([128, 9, OC], BF16)
        nc.sync.dma_start(out=wq_sb, in_=wq_t.rearrange("(kt p) m -> p kt m", p=128))
        wo_sb = singles.tile([HD, HPC, H], BF16)
        nc.sync.dma_start(out=wo_sb, in_=wo_t.rearrange("h d o -> d h o"))
        cs4 = singles.tile([128, nt, 288], F32)
        ident = singles.tile([128, 128], BF16)
        nc.sync.dma_start(out=ident, in_=idd)
        ebias = singles.tile([128, 1], F32)
        nc.vector.memset(ebias, EXP_BIAS)
        QK = singles.tile([HD, 4, T], BF16)
        vseg = singles.tile([128, nt, 194], BF16)   # per head: v(72) z(24) one(1)

        xts = {}

        def load_chunk(c0, cn, tls):
            xt = xin.tile([128, 9, 512], BF16, tag="xt", name=f"xt_{c0}")
            nc.sync.dma_start(
                out=xt[:, :, :cn],
                in_=x_t.rearrange("(kt p) t -> p kt t", p=128)[:, :, c0:c0 + cn])
            for (t0, tl) in tls:
                xts[t0] = (xt, t0 - c0)
                i = tidx[t0]
                nc.sync.dma_start(out=cs4[:, i, :], in_=cs4d[i])

        def qkv_mm(t0, tl):
            i = tidx[t0]
            ps = ps_qkv.tile([128, OC], F32, tag="psq", name=f"psq_{i}")
            xt, off = xts[t0]
            for kt in range(9):
                nc.tensor.matmul(ps[:tl, :], xt[:, kt, off:off + tl],
                                 wq_sb[:, kt, :], start=(kt == 0), stop=(kt == 8))
            return ps

        def rope_tp(t0, tl, ps):
            i = tidx[t0]
            qk = ps[:tl, 0:288].rearrange("p (j h d) -> p j h d", h=2, d=36)
            px1 = qk[:, :, 0, :]
            px2 = qk[:, :, 1, :]
            c = cs4[:tl, i, 0:144].rearrange("p (j d) -> p j d", d=36)
            s = cs4[:tl, i, 144:288].rearrange("p (j d) -> p j d", d=36)
            m1 = tmp.tile([128, 4, 36], F32, tag="m1", name=f"m1_{i}")
            m2 = tmp.tile([128, 4, 36], F32, tag="m2", name=f"m2_{i}")
            m3 = tmp.tile([128, 4, 36], F32, tag="m3", name=f"m3_{i}")
            m4 = tmp.tile([128, 4, 36], F32, tag="m4", name=f"m4_{i}")
            nc.vector.tensor_tensor(out=m1[:tl], in0=px1, in1=c, op=mybir.AluOpType.mult)
            nc.vector.tensor_tensor(out=m2[:tl], in0=px2, in1=s, op=mybir.AluOpType.mult)
            nc.vector.tensor_tensor(out=m3[:tl], in0=px2, in1=c, op=mybir.AluOpType.mult)
            nc.vector.tensor_tensor(out=m4[:tl], in0=px1, in1=s, op=mybir.AluOpType.mult)
            stg = stp.tile([128, 4, 2, 36], BF16, tag="stg", name=f"stg_{i}")
            nc.gpsimd.tensor_tensor(out=stg[:tl, :, 0, :], in0=m1[:tl], in1=m2[:tl],
                                    op=mybir.AluOpType.subtract)
            nc.gpsimd.tensor_tensor(out=stg[:tl, :, 1, :], in0=m3[:tl], in1=m4[:tl],
                                    op=mybir.AluOpType.add)
            pt = ps_tp.tile([HD, 512], BF16, tag="pt", name=f"pt_{i}")
            stgf = stg.rearrange("p j h d -> p (j h d)")
            for j in range(4):
                nc.tensor.transpose(pt[:, j * tl:(j + 1) * tl],
                                    stgf[:tl, j * 72:(j + 1) * 72], ident[:tl, :tl])
            nc.vector.tensor_copy(QK[:, :, t0:t0 + tl],
                                  pt[:, 0:4 * tl].rearrange("d (j t) -> d j t", j=4))
            nc.scalar.copy(vseg[:tl, i, 0:72], ps[:tl, 288:360])
            nc.scalar.copy(vseg[:tl, i, 97:169], ps[:tl, 360:432])
            nc.gpsimd.memset(vseg[:tl, i, 72:96], 0.0)
            nc.gpsimd.memset(vseg[:tl, i, 96:97], 1.0)
            nc.gpsimd.memset(vseg[:tl, i, 169:193], 0.0)
            nc.gpsimd.memset(vseg[:tl, i, 193:194], 1.0)

        pending = None
        for a, b, chunks in plan:
            for c0, cn, tls in chunks:
                load_chunk(c0, cn, tls)
                for (t0, tl) in tls:
                    ps = qkv_mm(t0, tl)
                    if pending is not None:
                        rope_tp(*pending)
                    pending = (t0, tl, ps)
        if pending is not None:
            rope_tp(*pending)

        # ---------------- phase 2: attention + out-proj ------------------
        def pv_do(cx, cn, st, h, first, last):
            s0, sn, es = st
            i = tidx[s0]
            nc.tensor.matmul(cx[:, :cn], vseg[:sn, i, h * 97:(h + 1) * 97],
                             es[:sn, :cn], start=first, stop=last)

        def attn_chunk(a, b, c0, cn):
            sts = []
            s0 = a
            while s0 < b:
                sn = min(128, b - s0)
                sts.append((s0, sn))
                s0 += sn
            ctxs = []
            for h in range(HPC):
                cx = ps_cx.tile([97, 512], F32, tag="cx", name=f"cx_{c0}_{h}")
                prev = None
                for si, (s0, sn) in enumerate(sts):
                    sc = ps_sc.tile([128, 512], F32, tag="sc", name=f"sc_{c0}_{h}_{si}")
                    nc.tensor.matmul(sc[:sn, :cn], QK[:, 2 + h, s0:s0 + sn],
                                     QK[:, h, c0:c0 + cn], start=True, stop=True)
                    es = esp.tile([128, 512], BF16, tag="es", name=f"es_{c0}_{h}_{si}")
                    nc.scalar.activation(es[:sn, :cn], sc[:sn, :cn],
                                         mybir.ActivationFunctionType.Exp,
                                         bias=ebias[:sn], scale=SCALE)
                    if prev is not None:
                        pv_do(cx, cn, prev, h, first=(si == 1), last=False)
                    prev = (s0, sn, es)
                pv_do(cx, cn, prev, h, first=(len(sts) == 1), last=True)
                rs = bcp.tile([1, 512], F32, tag="rs", name=f"rs_{c0}_{h}")
                nc.scalar.copy(rs[:, :cn], cx[96:97, :cn])
                rr = bcp.tile([1, 512], F32, tag="rr", name=f"rr_{c0}_{h}")
                nc.vector.reciprocal_approx_fast(out=rr[:, :cn], in_=rs[:, :cn])
                bc = bcp.tile([HD, 512], F32, tag="bc", name=f"bc_{c0}_{h}")
                nc.gpsimd.partition_broadcast(bc[:, :cn], rr[:, :cn])
                cxs = cxp.tile([HD, 512], BF16, tag="cxs", name=f"cxs_{c0}_{h}")
                nc.vector.tensor_tensor(out=cxs[:, :cn], in0=cx[0:HD, :cn],
                                        in1=bc[:, :cn], op=mybir.AluOpType.mult)
                ctxs.append(cxs)
            return ctxs

        def outproj(c0, cn, ctxs):
            for m in range(9):
                po = ps_ou.tile([128, 512], F32, tag="po", name=f"po_{c0}_{m}")
                for h in range(HPC):
                    nc.tensor.matmul(po[:, :cn], wo_sb[:, h, m * 128:(m + 1) * 128],
                                     ctxs[h][:, :cn], start=(h == 0), stop=(h == HPC - 1))
                ob = osb.tile([128, 512], F32, tag="ob", name=f"ob_{c0}_{m}")
                if m % 2 == 0:
                    nc.scalar.copy(ob[:, :cn], po[:, :cn])
                else:
                    nc.vector.tensor_copy(ob[:, :cn], po[:, :cn])
                nc.sync.dma_start(out=outT[m * 128:(m + 1) * 128, c0:c0 + cn],
                                  in_=ob[:, :cn])

        pend_out = None
        for a, b, chunks in plan:
            for c0, cn, tls in chunks:
                ctxs = attn_chunk(a, b, c0, cn)
                if pend_out is not None:
                    outproj(*pend_out)
                pend_out = (c0, cn, ctxs)
        if pend_out is not None:
            outproj(*pend_out)
    return nc


def _build_inputs(x, wqkv, wout, cos, sin, plan):
    tiles = _all_tiles(plan)
    nt = len(tiles)
    bf = ml_dtypes.bfloat16
    x_t = np.ascontiguousarray(x.T).astype(bf)
    c = cos[:, :HD2]
    s = sin[:, :HD2]
    cs4d = np.zeros((nt, 128, 288), np.float32)
    for i, (t0, tl) in enumerate(tiles):
        cs4d[i, :tl, 0:144] = np.tile(c[t0:t0 + tl], (1, 4))
        cs4d[i, :tl, 144:288] = np.tile(s[t0:t0 + tl], (1, 4))
    idd = np.eye(128, dtype=np.float32).astype(bf)

    in_maps = []
    for core in range(NCORES):
        h0 = core * HPC
        rows = []
        for kind in range(3):
            for h in range(HPC):
                base = kind * H + (h0 + h) * HD
                rows.extend(range(base, base + HD))
        wq = np.ascontiguousarray(wqkv[rows, :].T).astype(bf)      # [H, 432]
        cols = np.arange(h0 * HD, (h0 + HPC) * HD)
        wo = np.ascontiguousarray(wout[:, cols].T).astype(bf)      # [144, H]
        wo = np.ascontiguousarray(wo.reshape(HPC, HD, H))
        in_maps.append({"x_t": x_t, "wq_t": wq, "wo_t": wo,
                        "cs4d": cs4d, "idd": idd})
    return in_maps


def kernel(hidden_states, wqkv, bqkv, wout, bout, cos, sin, cu_seqlens,
           _trace=False):
    x = np.asarray(hidden_states, np.float32).reshape(T, H)
    plan = _plan(np.asarray(cu_seqlens).astype(np.int64))
    if plan not in _CACHE:
        nc = bacc.Bacc("TRN2", target_bir_lowering=False, debug=False)
        build(nc, plan)
        nc.compile()
        _CACHE[plan] = nc
    nc = _CACHE[plan]
    in_maps = _build_inputs(x, np.asarray(wqkv, np.float32),
                            np.asarray(wout, np.float32),
                            np.asarray(cos, np.float32),
                            np.asarray(sin, np.float32), plan)
    res = bass_utils.run_bass_kernel_spmd(nc, in_maps,
                                          core_ids=list(range(NCORES)),
                                          trace=_trace)
    out = np.zeros((H, T), np.float64)
    for core in range(NCORES):
        out += res.results[core]["outT"].astype(np.float64)
    out = out.T + np.asarray(bout, np.float64)[None, :]
    if _trace:
        kernel.last_exec_time_ns = res.exec_time_ns
        kernel.last_trace = res.instructions_and_trace
    return out.astype(np.float32).reshape(1, T, H)



# revision 6
# speedup vs baseline: 1.0391x; 1.0391x over previous
"""Fused varlen SigLIP attention block for TRN2, tensor-parallel over heads
across 8 NeuronCores (2 heads per core).

v2 layout (per core, heads 2c/2c+1):
  - host pre-transposes x -> xT [H, T] (bf16); qkvT per t-tile:
    psum[tl, 432] = xT_tile.T @ wqkvT (cols q0 q1 k0 k1 v0 v1).
  - rope: two contiguous DVE mults against tiled cos/sin tables
    (m1 = qk*C4, m2 = qk*S4), then two strided gpsimd combines
    (out1 = m1.h0 - m2.h1, out2 = m1.h1 + m2.h0).  gpsimd runs ONLY
    tensor_tensor all kernel (no library thrash).
  - PE-transpose q,k per 72-col quantity into QK [72, 4, T] bf16.
  - v evacuated s-major into vseg [tl, 128*2] (z-padded to 128 cols per
    head for FWL); ones column at local col 96 makes PV emit the
    softmax row-sum as ctx row 96.
  - attention per segment/chunk(<=512): both heads in lockstep
    (scoresT = kT.T@qT -> exp on ACT -> PV trails by 2 s-tiles);
    normalization: reciprocal (DVE) of psum rowsum row, broadcast via a
    K=1 fp32r ones-matmul on PE, DVE multiply into persistent ctxA.
  - out-proj at the end with stationary-weight reuse (h-outer,
    chunk-pair inner), evacuation split ACT/DVE into a [128, T] bf16
    staging tile, one batched DMA per 128-row block.  outT is bf16;
    host sums the 8 partials in f64.
  - ~32 identity warm-up matmuls at start keep the PE HAM warm while
    the first weight/activation DMAs land.
"""
import numpy as np
from collections import deque
from contextlib import ExitStack

import ml_dtypes
import concourse.bass as bass
import concourse.bacc as bacc
import concourse.tile as tile
import concourse.mybir as mybir
from concourse import bass_utils

F32 = mybir.dt.float32
F32R = mybir.dt.float32r
BF16 = mybir.dt.bfloat16

H = 1152
NH = 16
HD = 72
HD2 = 36
T = 4096
NCORES = 8
HPC = NH // NCORES          # heads per core
OC = 3 * HPC * HD           # 432
SCALE = HD ** -0.5
EXP_BIAS = -4.0

_CACHE = {}


def _plan(cu):
    bs = sorted(set([0, T] + [int(v) for v in cu[1:] if 0 < int(v) < T]))
    segs = [(a, b) for a, b in zip(bs[:-1], bs[1:]) if b > a]
    plan = []
    for (a, b) in segs:
        chunks = []
        c0 = a
        while c0 < b:
            cn = min(512, b - c0)
            tls = []
            t0 = c0
            while t0 < c0 + cn:
                tl = min(128, c0 + cn - t0)
                tls.append((t0, tl))
                t0 += tl
            chunks.append((c0, cn, tuple(tls)))
            c0 += cn
        plan.append((a, b, tuple(chunks)))
    return tuple(plan)


def _all_tiles(plan):
    out = []
    for a, b, chunks in plan:
        for c0, cn, tls in chunks:
            out.extend(tls)
    return out


def build(nc, plan):
    tiles = _all_tiles(plan)
    nt = len(tiles)
    tidx = {t0: i for i, (t0, tl) in enumerate(tiles)}

    x_t = nc.dram_tensor("x_t", [H, T], BF16, kind="ExternalInput").ap()
    wq_t = nc.dram_tensor("wq_t", [H, OC], BF16, kind="ExternalInput").ap()
    wo_t = nc.dram_tensor("wo_t", [HPC, HD, H], BF16, kind="ExternalInput").ap()
    cs4d = nc.dram_tensor("cs4d", [nt, 128, 576], F32, kind="ExternalInput").ap()
    idd = nc.dram_tensor("idd", [128, 128], BF16, kind="ExternalInput").ap()
    outT = nc.dram_tensor("outT", [H, T], BF16, kind="ExternalOutput").ap()

    with tile.TileContext(nc) as tc, ExitStack() as ctx:
        P = lambda **kw: ctx.enter_context(tc.tile_pool(**kw))
        singles = P(name="singles", bufs=1)
        xin = P(name="xin", bufs=2)
        cstp = P(name="cstp", bufs=4)
        tmp = P(name="tmp", bufs=2)
        stp = P(name="stp", bufs=3)
        esp = P(name="esp", bufs=4)
        bcp = P(name="bcp", bufs=2)
        osb = P(name="osb", bufs=2)
        ps_qkv = P(name="ps_qkv", bufs=2, space="PSUM")
        ps_tp = P(name="ps_tp", bufs=1, space="PSUM")
        ps_sc = P(name="ps_sc", bufs=3, space="PSUM")
        ps_cx = P(name="ps_cx", bufs=2, space="PSUM")

        ident = singles.tile([128, 128], BF16)
        nc.sync.dma_start(out=ident, in_=idd)

        # PE warm-up while the first weight/x DMAs are in flight
        for w in range(11):
            wm = ps_sc.tile([128, 512], F32, tag="sc", name=f"warm_{w}")
            for r in range(3):
                nc.tensor.matmul(wm[:, r * 128:(r + 1) * 128], ident, ident,
                                 start=True, stop=True)

        wq_sb = singles.tile([128, 9, OC], BF16)
        wq_r = wq_t.rearrange("(kt p) m -> p kt m", p=128)
        for kt in range(9):
            nc.sync.dma_start(out=wq_sb[:, kt, :], in_=wq_r[:, kt, :])

        ebias = singles.tile([128, 1], F32)
        nc.vector.memset(ebias, EXP_BIAS)
        ones = singles.tile([1, HD], BF16)
        nc.vector.memset(ones, 1.0)
        QK = singles.tile([HD, 4, T], BF16)
        vseg = singles.tile([128, nt, 256], BF16)
        nc.vector.memset(vseg, 0.0)
        nc.vector.memset(vseg[:, :, 96:97], 1.0)
        nc.vector.memset(vseg[:, :, 224:225], 1.0)
        ctxA = singles.tile([HD, HPC, T], BF16)
        wo_sb = singles.tile([HD, HPC, H], BF16)

        xts = {}
        csts = {}

        def load_chunk(c0, cn, tls):
            xt = xin.tile([128, 9, 512], BF16, tag="xt", name=f"xt_{c0}")
            nc.sync.dma_start(
                out=xt[:, :, :cn],
                in_=x_t.rearrange("(kt p) t -> p kt t", p=128)[:, :, c0:c0 + cn])
            for (t0, tl) in tls:
                xts[t0] = (xt, t0 - c0)
                i = tidx[t0]
                cst = cstp.tile([128, 576], F32, tag="cst", name=f"cst_{i}")
                nc.sync.dma_start(out=cst, in_=cs4d[i])
                csts[t0] = cst

        def qkv_mm(t0, tl):
            i = tidx[t0]
            ps = ps_qkv.tile([128, OC], F32, tag="psq", name=f"psq_{i}")
            xt, off = xts[t0]
            for kt in range(9):
                nc.tensor.matmul(ps[:tl, :], xt[:, kt, off:off + tl],
                                 wq_sb[:, kt, :], start=(kt == 0), stop=(kt == 8))
            return ps

        def rope_tp(t0, tl, ps):
            i = tidx[t0]
            cst = csts[t0]
            m1 = tmp.tile([128, 288], F32, tag="m1", name=f"m1_{i}")
            m2 = tmp.tile([128, 288], F32, tag="m2", name=f"m2_{i}")
            nc.vector.tensor_tensor(out=m1[:tl], in0=ps[:tl, 0:288],
                                    in1=cst[:tl, 0:288], op=mybir.AluOpType.mult)
            nc.vector.tensor_tensor(out=m2[:tl], in0=ps[:tl, 0:288],
                                    in1=cst[:tl, 288:576], op=mybir.AluOpType.mult)
            stg = stp.tile([128, 288], BF16, tag="stg", name=f"stg_{i}")
            m1v = m1.rearrange("p (j h d) -> p j h d", h=2, d=36)
            m2v = m2.rearrange("p (j h d) -> p j h d", h=2, d=36)
            sgv = stg.rearrange("p (j h d) -> p j h d", h=2, d=36)
            nc.gpsimd.tensor_tensor(out=sgv[:tl, :, 0, :], in0=m1v[:tl, :, 0, :],
                                    in1=m2v[:tl, :, 1, :],
                                    op=mybir.AluOpType.subtract)
            nc.gpsimd.tensor_tensor(out=sgv[:tl, :, 1, :], in0=m1v[:tl, :, 1, :],
                                    in1=m2v[:tl, :, 0, :], op=mybir.AluOpType.add)
            pt = ps_tp.tile([HD, 512], BF16, tag="pt", name=f"pt_{i}")
            for j in range(4):
                nc.tensor.transpose(pt[:, j * tl:(j + 1) * tl],
                                    stg[:tl, j * 72:(j + 1) * 72], ident[:tl, :tl])
            nc.vector.tensor_copy(QK[:, :, t0:t0 + tl],
                                  pt[:, 0:4 * tl].rearrange("d (j t) -> d j t", j=4))
            nc.vector.tensor_copy(out=vseg[:tl, i, 0:72], in_=ps[:tl, 288:360])
            nc.vector.tensor_copy(out=vseg[:tl, i, 128:200], in_=ps[:tl, 360:432])

        # ---------------- phase 1: qkv + rope ------------------
        pending = None
        for a, b, chunks in plan:
            for c0, cn, tls in chunks:
                load_chunk(c0, cn, tls)
                for (t0, tl) in tls:
                    ps = qkv_mm(t0, tl)
                    if pending is not None:
                        rope_tp(*pending)
                    pending = (t0, tl, ps)
        if pending is not None:
            rope_tp(*pending)

        # wout only needed for phase 3; keep it off the startup DMA path
        nc.sync.dma_start(out=wo_sb, in_=wo_t.rearrange("h d o -> d h o"))

        # ---------------- phase 2: attention ------------------
        def attn_chunk(a, b, c0, cn):
            sts = []
            s0 = a
            while s0 < b:
                sn = min(128, b - s0)
                sts.append((s0, sn))
                s0 += sn
            nst = len(sts)
            cxs = []
            for h in range(HPC):
                cx = ps_cx.tile([128, 512], F32, tag="cx", name=f"cx_{c0}_{h}")
                cxs.append(cx)
            q = [deque(), deque()]

            def pv(h, si, s0, sn, es):
                i = tidx[s0]
                nc.tensor.matmul(cxs[h][:, :cn], vseg[:sn, i, h * 128:(h + 1) * 128],
                                 es[:sn, :cn], start=(si == 0), stop=(si == nst - 1))

            for si, (s0, sn) in enumerate(sts):
                for h in range(HPC):
                    sc = ps_sc.tile([128, 512], F32, tag="sc",
                                    name=f"sc_{c0}_{h}_{si}")
                    nc.tensor.matmul(sc[:sn, :cn], QK[:, 2 + h, s0:s0 + sn],
                                     QK[:, h, c0:c0 + cn], start=True, stop=True)
                    es = esp.tile([128, 512], BF16, tag="es",
                                  name=f"es_{c0}_{h}_{si}")
                    nc.scalar.activation(es[:sn, :cn], sc[:sn, :cn],
                                         mybir.ActivationFunctionType.Exp,
                                         bias=ebias[:sn], scale=SCALE)
                    q[h].append((si, s0, sn, es))
                if si >= 1:
                    for h in range(HPC):
                        pv(h, *q[h].popleft())
            for h in range(HPC):
                while q[h]:
                    pv(h, *q[h].popleft())
            for h in range(HPC):
                rs = bcp.tile([1, 512], F32, tag="rs", name=f"rs_{c0}_{h}")
                nc.scalar.copy(rs[:, :cn], cxs[h][96:97, :cn])
                rr = bcp.tile([1, 512], F32, tag="rr", name=f"rr_{c0}_{h}")
                nc.vector.reciprocal_approx_fast(out=rr[:, :cn],
                                                 in_=rs[:, :cn])
                rrb = bcp.tile([1, 512], BF16, tag="rrb", name=f"rrb_{c0}_{h}")
                nc.vector.tensor_copy(out=rrb[:, :cn], in_=rr[:, :cn])
                bc = ps_sc.tile([128, 512], F32, tag="sc", name=f"bc_{c0}_{h}")
                nc.tensor.matmul(bc[:HD, :cn], ones, rrb[:, :cn],
                                 start=True, stop=True)
                bs = bcp.tile([HD, 512], F32, tag="bs", name=f"bs_{c0}_{h}")
                nc.vector.tensor_copy(out=bs[:, :cn], in_=bc[:HD, :cn])
                nc.vector.tensor_tensor(out=ctxA[:, h, c0:c0 + cn],
                                        in0=cxs[h][0:HD, :cn], in1=bs[:, :cn],
                                        op=mybir.AluOpType.mult)

        for a, b, chunks in plan:
            for c0, cn, tls in chunks:
                attn_chunk(a, b, c0, cn)

        # ---------------- phase 3: out-proj ------------------
        chunks_all = [(c0, cn) for a, b, chs in plan for (c0, cn, tls) in chs]
        for m in range(9):
            st = osb.tile([128, T], BF16, tag="stage", name=f"st_{m}")
            for g in range(0, len(chunks_all), 2):
                pair = chunks_all[g:g + 2]
                pos = [ps_cx.tile([128, 512], F32, tag="cx",
                                  name=f"po_{m}_{g}_{k}")
                       for k in range(len(pair))]
                for h in range(HPC):
                    for k, (c0, cn) in enumerate(pair):
                        nc.tensor.matmul(pos[k][:, :cn],
                                         wo_sb[:, h, m * 128:(m + 1) * 128],
                                         ctxA[:, h, c0:c0 + cn],
                                         start=(h == 0), stop=(h == HPC - 1),
                                         skip_group_check=True)
                for k, (c0, cn) in enumerate(pair):
                    if (g // 2 + k) % 2 == 0:
                        nc.scalar.copy(st[:, c0:c0 + cn], pos[k][:, :cn])
                    else:
                        nc.vector.tensor_copy(out=st[:, c0:c0 + cn],
                                              in_=pos[k][:, :cn])
            nc.sync.dma_start(out=outT[m * 128:(m + 1) * 128, :], in_=st)
    return nc


def _build_inputs(x, wqkv, wout, cos, sin, plan):
    tiles = _all_tiles(plan)
    nt = len(tiles)
    bf = ml_dtypes.bfloat16
    x_t = np.ascontiguousarray(x.T).astype(bf)
    c = cos[:, :HD2]
    s = sin[:, :HD2]
    cs4d = np.zeros((nt, 128, 576), np.float32)
    for i, (t0, tl) in enumerate(tiles):
        cs4d[i, :tl, 0:288] = np.tile(c[t0:t0 + tl], (1, 8))
        cs4d[i, :tl, 288:576] = np.tile(s[t0:t0 + tl], (1, 8))
    idd = np.eye(128, dtype=np.float32).astype(bf)

    in_maps = []
    for core in range(NCORES):
        h0 = core * HPC
        rows = []
        for kind in range(3):
            for h in range(HPC):
                base = kind * H + (h0 + h) * HD
                rows.extend(range(base, base + HD))
        wq = np.ascontiguousarray(wqkv[rows, :].T).astype(bf)      # [H, 432]
        cols = np.arange(h0 * HD, (h0 + HPC) * HD)
        wo = np.ascontiguousarray(wout[:, cols].T).astype(bf)      # [144, H]
        wo = np.ascontiguousarray(wo.reshape(HPC, HD, H))
        in_maps.append({"x_t": x_t, "wq_t": wq, "wo_t": wo,
                        "cs4d": cs4d, "idd": idd})
    return in_maps


def kernel(hidden_states, wqkv, bqkv, wout, bout, cos, sin, cu_seqlens,
           _trace=False):
    x = np.asarray(hidden_states, np.float32).reshape(T, H)
    plan = _plan(np.asarray(cu_seqlens).astype(np.int64))
    if plan not in _CACHE:
        nc = bacc.Bacc("TRN2", target_bir_lowering=False, debug=False)
        build(nc, plan)
        nc.compile()
        _CACHE[plan] = nc
    nc = _CACHE[plan]
    in_maps = _build_inputs(x, np.asarray(wqkv, np.float32),
                            np.asarray(wout, np.float32),
                            np.asarray(cos, np.float32),
                            np.asarray(sin, np.float32), plan)
    res = bass_utils.run_bass_kernel_spmd(nc, in_maps,
                                          core_ids=list(range(NCORES)),
                                          trace=_trace)
    out = np.zeros((H, T), np.float64)
    for core in range(NCORES):
        out += res.results[core]["outT"].astype(np.float64)
    out = out.T + np.asarray(bout, np.float64)[None, :]
    if _trace:
        kernel.last_exec_time_ns = res.exec_time_ns
        kernel.last_trace = res.instructions_and_trace
    return out.astype(np.float32).reshape(1, T, H)


# revision 8
# speedup vs baseline: 1.1265x; 1.0841x over previous
"""Fused varlen SigLIP attention block for TRN2, tensor-parallel over heads
across 8 NeuronCores (2 heads per core).

v3: same math as v2 but with fully zippered emission — per-tile qkv/rope
steps, per-si attention steps (both heads in lockstep), and per-pair
out-proj steps are interleaved in one stream so the PE never sits behind
an ACT-bound stretch (exp) or a cold-clock tail.

  - qkvT per t-tile: psum[tl, 432] = xT_tile.T @ wqkvT; xT loaded per-kt
    for fast startup.  ~24 identity warm-up matmuls cover the first DMAs.
  - rope: two contiguous DVE mults against tiled cos/sin tables, two
    strided gpsimd combines (gpsimd runs only tensor_tensor all kernel).
  - PE-transpose q,k into QK [72, 4, T] bf16; v into vseg [tl, 2*128]
    (z-padded per head, ones col at local 96 -> PV row 96 = softmax sum).
  - attention per chunk: scores -> exp(ACT) -> PV trailing by 1 si;
    normalization: ACT copy of rowsum row, DVE reciprocal, K=1 bf16
    ones-matmul broadcast, DVE multiply into persistent ctxA bf16.
  - out-proj per chunk-pair per m-block (po bufs=1, h-accumulated), DVE
    evacuation into [128, 1024] bf16 tiles, one DMA per (pair, m).
  - outT bf16; host sums the 8 partials in f64.
"""
import numpy as np
from collections import deque
from contextlib import ExitStack

import ml_dtypes
import concourse.bass as bass
import concourse.bacc as bacc
import concourse.tile as tile
import concourse.mybir as mybir
from concourse import bass_utils

F32 = mybir.dt.float32
BF16 = mybir.dt.bfloat16

H = 1152
NH = 16
HD = 72
HD2 = 36
T = 4096
NCORES = 8
HPC = NH // NCORES          # heads per core
OC = 3 * HPC * HD           # 432
SCALE = HD ** -0.5
EXP_BIAS = -4.0

_CACHE = {}


def _plan(cu):
    bs = sorted(set([0, T] + [int(v) for v in cu[1:] if 0 < int(v) < T]))
    segs = [(a, b) for a, b in zip(bs[:-1], bs[1:]) if b > a]
    plan = []
    for (a, b) in segs:
        chunks = []
        c0 = a
        while c0 < b:
            cn = min(512, b - c0)
            tls = []
            t0 = c0
            while t0 < c0 + cn:
                tl = min(128, c0 + cn - t0)
                tls.append((t0, tl))
                t0 += tl
            chunks.append((c0, cn, tuple(tls)))
            c0 += cn
        plan.append((a, b, tuple(chunks)))
    return tuple(plan)


def _all_tiles(plan):
    out = []
    for a, b, chunks in plan:
        for c0, cn, tls in chunks:
            out.extend(tls)
    return out


def build(nc, plan):
    tiles = _all_tiles(plan)
    nt = len(tiles)
    tidx = {t0: i for i, (t0, tl) in enumerate(tiles)}

    x_t = nc.dram_tensor("x_t", [H, T], BF16, kind="ExternalInput").ap()
    wq_t = nc.dram_tensor("wq_t", [H, OC], BF16, kind="ExternalInput").ap()
    wo_t = nc.dram_tensor("wo_t", [HPC, HD, H], BF16, kind="ExternalInput").ap()
    cs4d = nc.dram_tensor("cs4d", [nt, 128, 576], F32, kind="ExternalInput").ap()
    idd = nc.dram_tensor("idd", [128, 128], BF16, kind="ExternalInput").ap()
    outT = nc.dram_tensor("outT", [H, T], BF16, kind="ExternalOutput").ap()

    with tile.TileContext(nc) as tc, ExitStack() as ctx:
        P = lambda **kw: ctx.enter_context(tc.tile_pool(**kw))
        singles = P(name="singles", bufs=1)
        xin = P(name="xin", bufs=2)
        cstp = P(name="cstp", bufs=4)
        tmp = P(name="tmp", bufs=2)
        stp = P(name="stp", bufs=3)
        esp = P(name="esp", bufs=6)
        bcp = P(name="bcp", bufs=3)
        obp = P(name="obp", bufs=3)
        ps_qkv = P(name="ps_qkv", bufs=2, space="PSUM")
        ps_tp = P(name="ps_tp", bufs=1, space="PSUM")
        ps_sc = P(name="ps_sc", bufs=2, space="PSUM")
        ps_cx = P(name="ps_cx", bufs=2, space="PSUM")
        ps_po = P(name="ps_po", bufs=1, space="PSUM")

        ident = singles.tile([128, 128], BF16)
        nc.sync.dma_start(out=ident, in_=idd)

        # PE warm-up while the first weight/x DMAs are in flight
        for w in range(8):
            wm = ps_sc.tile([128, 512], F32, tag="sc", name=f"warm_{w}")
            for r in range(3):
                nc.tensor.matmul(wm[:, r * 128:(r + 1) * 128], ident, ident,
                                 start=True, stop=True)

        wq_sb = singles.tile([128, 9, OC], BF16)
        wq_r = wq_t.rearrange("(kt p) m -> p kt m", p=128)
        for kt in range(9):
            nc.sync.dma_start(out=wq_sb[:, kt, :], in_=wq_r[:, kt, :])

        ebias = singles.tile([128, 1], F32)
        nc.vector.memset(ebias, EXP_BIAS)
        ones = singles.tile([1, HD], BF16)
        nc.vector.memset(ones, 1.0)
        QK = singles.tile([HD, 4, T], BF16)
        vseg = singles.tile([128, nt, 256], BF16)
        nc.vector.memset(vseg, 0.0)
        nc.vector.memset(vseg[:, :, 96:97], 1.0)
        nc.vector.memset(vseg[:, :, 224:225], 1.0)
        ctxA = singles.tile([HD, HPC, T], BF16)
        wo_sb = singles.tile([HD, HPC, H], BF16)

        xts = {}
        csts = {}
        x_r = x_t.rearrange("(kt p) t -> p kt t", p=128)

        def load_chunk(c0, cn, tls):
            xt = xin.tile([128, 9, 512], BF16, tag="xt", name=f"xt_{c0}")
            for kt in range(9):
                nc.sync.dma_start(out=xt[:, kt, :cn],
                                  in_=x_r[:, kt, c0:c0 + cn])
            for (t0, tl) in tls:
                xts[t0] = (xt, t0 - c0)
                i = tidx[t0]
                cst = cstp.tile([128, 576], F32, tag="cst", name=f"cst_{i}")
                nc.sync.dma_start(out=cst, in_=cs4d[i])
                csts[t0] = cst

        def qkv_mm(t0, tl):
            i = tidx[t0]
            ps = ps_qkv.tile([128, OC], F32, tag="psq", name=f"psq_{i}")
            xt, off = xts[t0]
            for kt in range(9):
                nc.tensor.matmul(ps[:tl, :], xt[:, kt, off:off + tl],
                                 wq_sb[:, kt, :], start=(kt == 0), stop=(kt == 8))
            return ps

        def rope_tp(t0, tl, ps):
            i = tidx[t0]
            cst = csts.pop(t0)
            m1 = tmp.tile([128, 288], F32, tag="m1", name=f"m1_{i}")
            m2 = tmp.tile([128, 288], F32, tag="m2", name=f"m2_{i}")
            nc.vector.tensor_tensor(out=m1[:tl], in0=ps[:tl, 0:288],
                                    in1=cst[:tl, 0:288], op=mybir.AluOpType.mult)
            nc.vector.tensor_tensor(out=m2[:tl], in0=ps[:tl, 0:288],
                                    in1=cst[:tl, 288:576], op=mybir.AluOpType.mult)
            stg = stp.tile([128, 288], BF16, tag="stg", name=f"stg_{i}")
            m1v = m1.rearrange("p (j h d) -> p j h d", h=2, d=36)
            m2v = m2.rearrange("p (j h d) -> p j h d", h=2, d=36)
            sgv = stg.rearrange("p (j h d) -> p j h d", h=2, d=36)
            nc.gpsimd.tensor_tensor(out=sgv[:tl, :, 0, :], in0=m1v[:tl, :, 0, :],
                                    in1=m2v[:tl, :, 1, :],
                                    op=mybir.AluOpType.subtract)
            nc.gpsimd.tensor_tensor(out=sgv[:tl, :, 1, :], in0=m1v[:tl, :, 1, :],
                                    in1=m2v[:tl, :, 0, :], op=mybir.AluOpType.add)
            pt = ps_tp.tile([HD, 512], BF16, tag="pt", name=f"pt_{i}")
            for j in range(4):
                nc.tensor.transpose(pt[:, j * tl:(j + 1) * tl],
                                    stg[:tl, j * 72:(j + 1) * 72], ident[:tl, :tl])
            nc.vector.tensor_copy(QK[:, :, t0:t0 + tl],
                                  pt[:, 0:4 * tl].rearrange("d (j t) -> d j t", j=4))
            nc.vector.tensor_copy(out=vseg[:tl, i, 0:72], in_=ps[:tl, 288:360])
            nc.vector.tensor_copy(out=vseg[:tl, i, 128:200], in_=ps[:tl, 360:432])

        # ---------- attention chunk as a sequence of small steps ----------
        class AttnChunk:
            def __init__(self, a, b, c0, cn):
                self.c0, self.cn = c0, cn
                sts = []
                s0 = a
                while s0 < b:
                    sn = min(128, b - s0)
                    sts.append((s0, sn))
                    s0 += sn
                self.sts = sts
                self.q = [deque(), deque()]
                self.cxs = None

            def _pv(self, h, si, s0, sn, es):
                i = tidx[s0]
                nc.tensor.matmul(self.cxs[h][:, :self.cn],
                                 vseg[:sn, i, h * 128:(h + 1) * 128],
                                 es[:sn, :self.cn],
                                 start=(si == 0), stop=(si == len(self.sts) - 1))

            def step(self, si):
                c0, cn = self.c0, self.cn
                if si == 0:
                    self.cxs = [ps_cx.tile([128, 512], F32, tag="cx",
                                           name=f"cx_{c0}_{h}")
                                for h in range(HPC)]
                s0, sn = self.sts[si]
                for h in range(HPC):
                    sc = ps_sc.tile([128, 512], F32, tag="sc",
                                    name=f"sc_{c0}_{h}_{si}")
                    nc.tensor.matmul(sc[:sn, :cn], QK[:, 2 + h, s0:s0 + sn],
                                     QK[:, h, c0:c0 + cn], start=True, stop=True)
                    es = esp.tile([128, 512], BF16, tag="es",
                                  name=f"es_{c0}_{h}_{si}")
                    nc.scalar.activation(es[:sn, :cn], sc[:sn, :cn],
                                         mybir.ActivationFunctionType.Exp,
                                         bias=ebias[:sn], scale=SCALE)
                    self.q[h].append((si, s0, sn, es))
                if si >= 1:
                    for h in range(HPC):
                        self._pv(h, *self.q[h].popleft())

            def finalize(self):
                c0, cn = self.c0, self.cn
                for h in range(HPC):
                    while self.q[h]:
                        self._pv(h, *self.q[h].popleft())
                for h in range(HPC):
                    rs = bcp.tile([1, 512], F32, tag="rs", name=f"rs_{c0}_{h}")
                    nc.scalar.copy(rs[:, :cn], self.cxs[h][96:97, :cn])
                    rr = bcp.tile([1, 512], F32, tag="rr", name=f"rr_{c0}_{h}")
                    nc.vector.reciprocal_approx_fast(out=rr[:, :cn],
                                                     in_=rs[:, :cn])
                    rrb = bcp.tile([1, 512], BF16, tag="rrb",
                                   name=f"rrb_{c0}_{h}")
                    nc.vector.tensor_copy(out=rrb[:, :cn], in_=rr[:, :cn])
                    bc = ps_sc.tile([128, 512], F32, tag="sc",
                                    name=f"bc_{c0}_{h}")
                    nc.tensor.matmul(bc[:HD, :cn], ones, rrb[:, :cn],
                                     start=True, stop=True)
                    bs = bcp.tile([HD, 512], F32, tag="bs", name=f"bs_{c0}_{h}")
                    nc.vector.tensor_copy(out=bs[:, :cn], in_=bc[:HD, :cn])
                    nc.vector.tensor_tensor(out=ctxA[:, h, c0:c0 + cn],
                                            in0=self.cxs[h][0:HD, :cn],
                                            in1=bs[:, :cn],
                                            op=mybir.AluOpType.mult)

        # ---------- out-proj step: one (pair, m) ----------
        def outp_step(pair, pi, m):
            base = pair[0][0]
            wide = pair[-1][0] + pair[-1][1] - base
            ob = obp.tile([128, 1024], BF16, tag="ob", name=f"ob_{pi}_{m}")
            for (c0, cn) in pair:
                po = ps_po.tile([128, 512], F32, tag="po",
                                name=f"po_{m}_{c0}")
                for h in range(HPC):
                    nc.tensor.matmul(po[:, :cn],
                                     wo_sb[:, h, m * 128:(m + 1) * 128],
                                     ctxA[:, h, c0:c0 + cn],
                                     start=(h == 0), stop=(h == HPC - 1))
                nc.vector.tensor_copy(out=ob[:, c0 - base:c0 - base + cn],
                                      in_=po[:, :cn])
            nc.sync.dma_start(out=outT[m * 128:(m + 1) * 128, base:base + wide],
                              in_=ob[:, :wide])

        # ---------- build step lists ----------
        p1_steps = []          # (seg_idx, step descriptor)
        chunk_list = []
        for sidx, (a, b, chunks) in enumerate(plan):
            for c0, cn, tls in chunks:
                chunk_list.append((c0, cn))
                p1_steps.append((sidx, ('load', c0, cn, tls)))
                for (t0, tl) in tls:
                    p1_steps.append((sidx, ('tile', t0, tl)))

        pairs = [chunk_list[i:i + 2] for i in range(0, len(chunk_list), 2)]
        pair_ready_after = {}  # c0 of last chunk in pair -> pair index
        for pi, pr in enumerate(pairs):
            pair_ready_after[pr[-1][0]] = pi

        # attn steps with completion metadata (c0 on the finalize step)
        attn_by_seg = []
        for sidx, (a, b, chunks) in enumerate(plan):
            seg_attn = []
            for c0, cn, tls in chunks:
                ac = AttnChunk(a, b, c0, cn)
                for si in range(len(ac.sts)):
                    seg_attn.append((None, lambda ac=ac, si=si: ac.step(si)))
                seg_attn.append((c0, lambda ac=ac: ac.finalize()))
            attn_by_seg.append(seg_attn)

        pending = [None]

        def emit_p1(step):
            kind = step[1][0]
            if kind == 'load':
                _, c0, cn, tls = step[1]
                load_chunk(c0, cn, tls)
            else:
                _, t0, tl = step[1]
                ps = qkv_mm(t0, tl)
                if pending[0] is not None:
                    rope_tp(*pending[0])
                pending[0] = (t0, tl, ps)

        ready_attn = deque()
        ready_outp = deque()
        done_p1_segs = 0

        def on_attn_emitted(meta):
            if meta is None:
                return
            c0 = meta
            pi = pair_ready_after.get(c0)
            if pi is not None:
                for m in range(9):
                    ready_outp.append(lambda pi=pi, m=m:
                                      outp_step(pairs[pi], pi, m))

        wo_loaded = [False]
        cur_seg = 0
        for idx, step in enumerate(p1_steps):
            sidx = step[0]
            if sidx > cur_seg:
                ready_attn.extend(attn_by_seg[cur_seg])
                cur_seg = sidx
                if not wo_loaded[0]:
                    nc.sync.dma_start(out=wo_sb,
                                      in_=wo_t.rearrange("h d o -> d h o"))
                    wo_loaded[0] = True
            emit_p1(step)
            for _ in range(3):
                if ready_attn:
                    meta, fn = ready_attn.popleft()
                    fn()
                    on_attn_emitted(meta)
            if ready_outp:
                ready_outp.popleft()()
        # flush rope of last tile
        if pending[0] is not None:
            rope_tp(*pending[0])
            pending[0] = None
        ready_attn.extend(attn_by_seg[cur_seg])
        if not wo_loaded[0]:
            nc.sync.dma_start(out=wo_sb, in_=wo_t.rearrange("h d o -> d h o"))
        # drain
        while ready_attn or ready_outp:
            for _ in range(2):
                if ready_attn:
                    meta, fn = ready_attn.popleft()
                    fn()
                    on_attn_emitted(meta)
            if ready_outp:
                ready_outp.popleft()()
    return nc


def _build_inputs(x, wqkv, wout, cos, sin, plan):
    tiles = _all_tiles(plan)
    nt = len(tiles)
    bf = ml_dtypes.bfloat16
    x_t = np.ascontiguousarray(x.T).astype(bf)
    c = cos[:, :HD2]
    s = sin[:, :HD2]
    cs4d = np.zeros((nt, 128, 576), np.float32)
    for i, (t0, tl) in enumerate(tiles):
        cs4d[i, :tl, 0:288] = np.tile(c[t0:t0 + tl], (1, 8))
        cs4d[i, :tl, 288:576] = np.tile(s[t0:t0 + tl], (1, 8))
    idd = np.eye(128, dtype=np.float32).astype(bf)

    in_maps = []
    for core in range(NCORES):
        h0 = core * HPC
        rows = []
        for kind in range(3):
            for h in range(HPC):
                base = kind * H + (h0 + h) * HD
                rows.extend(range(base, base + HD))
        wq = np.ascontiguousarray(wqkv[rows, :].T).astype(bf)      # [H, 432]
        cols = np.arange(h0 * HD, (h0 + HPC) * HD)
        wo = np.ascontiguousarray(wout[:, cols].T).astype(bf)      # [144, H]
        wo = np.ascontiguousarray(wo.reshape(HPC, HD, H))
        in_maps.append({"x_t": x_t, "wq_t": wq, "wo_t": wo,
                        "cs4d": cs4d, "idd": idd})
    return in_maps


def kernel(hidden_states, wqkv, bqkv, wout, bout, cos, sin, cu_seqlens,
           _trace=False):
    x = np.asarray(hidden_states, np.float32).reshape(T, H)
    plan = _plan(np.asarray(cu_seqlens).astype(np.int64))
    if plan not in _CACHE:
        nc = bacc.Bacc("TRN2", target_bir_lowering=False, debug=False)
        build(nc, plan)
        nc.compile()
        _CACHE[plan] = nc
    nc = _CACHE[plan]
    in_maps = _build_inputs(x, np.asarray(wqkv, np.float32),
                            np.asarray(wout, np.float32),
                            np.asarray(cos, np.float32),
                            np.asarray(sin, np.float32), plan)
    res = bass_utils.run_bass_kernel_spmd(nc, in_maps,
                                          core_ids=list(range(NCORES)),
                                          trace=_trace)
    out = np.zeros((H, T), np.float64)
    for core in range(NCORES):
        out += res.results[core]["outT"].astype(np.float64)
    out = out.T + np.asarray(bout, np.float64)[None, :]
    if _trace:
        kernel.last_exec_time_ns = res.exec_time_ns
        kernel.last_trace = res.instructions_and_trace
    return out.astype(np.float32).reshape(1, T, H)
